# revision 20
# baseline (speedup 1.0000x reference)
"""Trainium2 Bass kernel for nn_CrossAttention_71073118814901.

Reference computation (per branch r, batch b, with N = H*W = 4096, d = 32):
    q = wq_r @ x1[b] + bq_r            (32, N)
    k = wk_r @ x2[b] + bk_r            (32, N)
    v = wv_r @ x2[b] + bv_r            (256, N)
    energy = q^T k                     (N, N)
    attn = softmax(energy, axis=-1)
    out_rb = v @ attn^T                (256, N)
    final[b] = x1[b] + x2[b] + out_1b + out_2b

Sharding: 8 (branch, batch) pairs -> 8 NeuronCores, fully data parallel.
Core i handles branch (i // 4) and batch (i % 4).  The final x1+x2+out1+out2
combination happens on the host during unsharding.

Device algorithm per core (all matmul operands bf16, f32 PSUM accumulation):

  E^T(j, i) = sum_d K(d, j) Q(d, i)  (K=32-contraction matmuls)
  S^T = exp(E^T)  on ScalarE, free dim 1024 (2 j-blocks per activation),
      no max subtraction (|energy| < ~6 at this model's scale)
  Vt(j, c) = sum_c' x2(c', j) wv^T(c', c), stored with a fused ones column:
      rhs_j = [Vt(j, :) | 1]  (128 x 257)
  out^T(i, c|den) = sum_j S^T(j, i-chunk)^T @ rhs_j
      - S^T chunks are the *stationary* operand, so the softmax denominator
        comes out as column 256 of the same accumulation (no separate
        ones-matmul), and the output lands i-on-partitions.
  out^T(i, c) = out^T(i, c) * recip(den(i)) + bv(c)   (single fused DVE op)

The host transposes each core's (N, C) result back to (C, H, W).
"""

import os
import sys

import numpy as np

if "/opt/trn_rl_repo" not in sys.path:
    sys.path.insert(0, "/opt/trn_rl_repo")

import concourse.bass as bass
import concourse.tile as tile
from concourse import mybir
from concourse.bass_utils import run_bass_kernel_spmd

try:  # pragma: no cover
    import antenv.axon_hooks  # noqa: F401
except ImportError:
    # Containers whose antenv stub lacks axon_hooks crash inside
    # run_bass_kernel_spmd when BASS_TRACE=1.  Register a no-op hook module
    # so tracing degrades gracefully (bass_utils skips the trace).
    import types as _types

    _hooks = _types.ModuleType("antenv.axon_hooks")
    _hooks.get_axon_ntff_profile_hook = lambda: None
    sys.modules["antenv.axon_hooks"] = _hooks

F32 = mybir.dt.float32
BF16 = mybir.dt.bfloat16

B, C, H, W = 4, 256, 64, 64
N = H * W            # 4096
D = 32               # query/key channels
P = 128              # SBUF partitions
NCH = C // P         # 2 channel chunks
NJ = N // P          # 32 key-position chunks
CV = C + 1           # value channels + fused ones column
I_TILE = 512         # output columns per tile (4 i-chunks of 128)
NI = N // I_TILE     # 8
IC = I_TILE // P     # 4 i-chunks per tile
JG = 2               # j-blocks per exp group (free dim 1024 activations)
NG = NJ // JG        # 16 groups

_ctr = [0]


def _fix_multi_waits(nc):
    """This container's walrus build rejects more than one sync-wait per
    instruction.  Hoist all but one wait of each multi-wait instruction onto
    same-engine NOPs inserted immediately before it (same sequencer => same
    blocking semantics)."""
    for f in nc.m.functions:
        for bb in f.blocks:
            il = bb.instructions
            i = 0
            while i < len(il):
                inst = il[i]
                si = inst.sync_info
                if si is not None and len(si.on_wait) > 1:
                    waits = list(si.on_wait)
                    inst.sync_info = mybir.SyncInfo(
                        on_wait=[waits[-1]], on_update=list(si.on_update)
                    )
                    for w in waits[:-1]:
                        _ctr[0] += 1
                        nop = mybir.InstNoOp(
                            name=f"waitfix-{_ctr[0]}",
                            ins=[],
                            outs=[],
                            engine=inst.engine,
                        )
                        nop.sync_info = mybir.SyncInfo(on_wait=[w], on_update=[])
                        il.insert(i, nop)
                        i += 1
                i += 1


def _build_nc():
    nc = bass.Bass()

    xq_d = nc.declare_dram_parameter("xq", [C, N], BF16, isOutput=False)
    xkv_d = nc.declare_dram_parameter("xkv", [C, N], BF16, isOutput=False)
    wqT_d = nc.declare_dram_parameter("wqT", [C, D], BF16, isOutput=False)
    wkT_d = nc.declare_dram_parameter("wkT", [C, D], BF16, isOutput=False)
    wvT_d = nc.declare_dram_parameter("wvT", [C, C], BF16, isOutput=False)
    bq_d = nc.declare_dram_parameter("bq", [D, 1], F32, isOutput=False)
    bk_d = nc.declare_dram_parameter("bk", [D, 1], F32, isOutput=False)
    bv_d = nc.declare_dram_parameter("bv", [1, C], F32, isOutput=False)
    outT_d = nc.declare_dram_parameter("outT", [N, C], F32, isOutput=True)

    Exp = mybir.ActivationFunctionType.Exp
    mult = mybir.AluOpType.mult
    add = mybir.AluOpType.add

    with tile.TileContext(nc) as tc:
        with (
            tc.tile_pool(name="const", bufs=1) as const,
            tc.tile_pool(name="xbuf", bufs=1) as xbuf,
            tc.tile_pool(name="qk", bufs=1) as qkpool,
            tc.tile_pool(name="vt", bufs=1) as vtpool,
            tc.tile_pool(name="spool", bufs=4) as spool,
            tc.tile_pool(name="epi", bufs=3) as epi,
        ):
            # ---- constants -------------------------------------------------
            wqT_t = const.tile([P, NCH, D], BF16)
            wkT_t = const.tile([P, NCH, D], BF16)
            wvT_t = const.tile([P, NCH, C], BF16)
            nc.sync.dma_start(
                out=wqT_t[:], in_=wqT_d.rearrange("(h p) d -> p h d", p=P)
            )
            nc.sync.dma_start(
                out=wkT_t[:], in_=wkT_d.rearrange("(h p) d -> p h d", p=P)
            )
            nc.sync.dma_start(
                out=wvT_t[:], in_=wvT_d.rearrange("(h p) c -> p h c", p=P)
            )
            bq_t = const.tile([D, 1], F32)
            bk_t = const.tile([D, 1], F32)
            nc.sync.dma_start(out=bq_t[:], in_=bq_d[:])
            nc.sync.dma_start(out=bk_t[:], in_=bk_d[:])
            # bv broadcast to all partitions once (free-dim = value channel)
            bvb_t = const.tile([P, C], F32)
            bv_ap = bv_d[:]
            bv_bcast_src = bass.AP(
                tensor=bv_ap.tensor, offset=bv_ap.offset, ap=[[0, P]] + list(bv_ap.ap)[1:]
            )
            nc.sync.dma_start(out=bvb_t[:], in_=bv_bcast_src)
            # prime the exp table-set load so it overlaps the input DMAs
            warm_t = const.tile([1, 1], F32)
            nc.vector.memset(warm_t[:], 0.0)
            warm2_t = const.tile([1, 1], F32)
            nc.scalar.activation(out=warm2_t[:], in_=warm_t[:], func=Exp)

            # ---- load x (separate tiles per 512-column slice: Tile's dep
            # tracking is whole-tile, so per-slice tiles let the prologue
            # matmuls start as soon as their own slice has landed) ----------
            XD = 512
            NX = N // XD
            xq_ts = [
                xbuf.tile([P, NCH, XD], BF16, name=f"xq{s}") for s in range(NX)
            ]
            xkv_ts = [
                xbuf.tile([P, NCH, XD], BF16, name=f"xkv{s}") for s in range(NX)
            ]
            dma_engines = [nc.sync, nc.gpsimd, nc.scalar]
            for s in range(NX):
                xl = slice(s * XD, (s + 1) * XD)
                for h in range(NCH):
                    dma_engines[s % 3].dma_start(
                        out=xq_ts[s][:, h, :], in_=xq_d[h * P : (h + 1) * P, xl]
                    )
                    dma_engines[s % 3].dma_start(
                        out=xkv_ts[s][:, h, :], in_=xkv_d[h * P : (h + 1) * P, xl]
                    )

            # ---- Q, K: strip 0 of a 4x-replicated (128, N) layout ---------
            ps_pre_cm = tc.tile_pool(name="ps_pre", bufs=2, space="PSUM")
            ps_pre = ps_pre_cm.__enter__()
            q_t = qkpool.tile([D, N], BF16)
            k_t = qkpool.tile([D, N], BF16)
            for it in range(NX):
                sl = slice(it * XD, (it + 1) * XD)
                pq = ps_pre.tile([D, XD], F32)
                for h in range(NCH):
                    nc.tensor.matmul(
                        pq[:], wqT_t[:, h, :], xq_ts[it][:, h, :],
                        start=(h == 0), stop=(h == NCH - 1),
                    )
                nc.vector.tensor_scalar_add(q_t[:, sl], pq[:], bq_t[:])
                pk = ps_pre.tile([D, XD], F32)
                for h in range(NCH):
                    nc.tensor.matmul(
                        pk[:], wkT_t[:, h, :], xkv_ts[it][:, h, :],
                        start=(h == 0), stop=(h == NCH - 1),
                    )
                nc.vector.tensor_scalar_add(k_t[:, sl], pk[:], bk_t[:])

            # ---- Vt(j, c) with fused ones column --------------------------
            vt_t = vtpool.tile([P, NJ, CV], BF16)
            nc.vector.memset(vt_t[:, :, C : C + 1], 1.0)
            JPX = XD // P
            for j in range(NJ):
                jo = (j % JPX) * P
                pv = ps_pre.tile([P, C], F32)
                for h in range(NCH):
                    nc.tensor.matmul(
                        pv[:], xkv_ts[j // JPX][:, h, jo : jo + P], wvT_t[:, h, :],
                        start=(h == 0), stop=(h == NCH - 1),
                    )
                nc.vector.tensor_copy(vt_t[:, j, 0:C], pv[:])

            ps_pre_cm.__exit__(None, None, None)

            # ---- attention main loop --------------------------------------
            ps_e_cm = tc.tile_pool(name="ps_e", bufs=2, space="PSUM")
            ps_o_cm = tc.tile_pool(name="ps_o", bufs=1, space="PSUM")
            ps_e = ps_e_cm.__enter__()
            ps_o = ps_o_cm.__enter__()
            for it in range(NI):
                sl = slice(it * I_TILE, (it + 1) * I_TILE)
                po = [
                    ps_o.tile([P, CV], F32, tag=f"po{ic}", name=f"po{ic}")
                    for ic in range(IC)
                ]

                def emit_qk_exp(g, sl=sl):
                    pe2 = ps_e.tile([P, JG, I_TILE], F32, name="pe2")
                    for r in range(JG):
                        j = g * JG + r
                        nc.tensor.matmul(
                            pe2[:, r, :],
                            k_t[:, j * P : (j + 1) * P],
                            q_t[:, sl],
                            start=True,
                            stop=True,
                        )
                    s4 = spool.tile([P, JG, I_TILE], BF16, name="s4")
                    nc.scalar.activation(out=s4[:], in_=pe2[:], func=Exp)
                    return s4

                # QK/exp run two groups ahead of their AV consumers, but the
                # refill for group g+2 is emitted AFTER AV(g): the PE queue is
                # strict FIFO and QK(g+2) blocks on exp(g) freeing its PSUM
                # slot, so it must sit behind AV(g) in the stream.
                s4q = {0: emit_qk_exp(0), 1: emit_qk_exp(1)}
                for g in range(NG):
                    s4 = s4q.pop(g)
                    for r in range(JG):
                        j = g * JG + r
                        first, last = (j == 0), (j == NJ - 1)
                        for ic in range(IC):
                            nc.tensor.matmul(
                                po[ic][:],
                                s4[:, r, ic * P : (ic + 1) * P],
                                vt_t[:, j, :],
                                start=first,
                                stop=last,
                            )
                    if g + 2 < NG:
                        s4q[g + 2] = emit_qk_exp(g + 2)
                # epilogue: per i-chunk divide by denominator, add bv
                for ic in range(IC):
                    recip_t = epi.tile([P, 1], F32, tag="recip")
                    nc.vector.reciprocal(recip_t[:], po[ic][:, C : C + 1])
                    o_t = epi.tile([P, C], F32, tag="o")
                    nc.vector.scalar_tensor_tensor(
                        out=o_t[:],
                        in0=po[ic][:, 0:C],
                        scalar=recip_t[:],
                        in1=bvb_t[:],
                        op0=mult,
                        op1=add,
                    )
                    row = it * I_TILE + ic * P
                    nc.sync.dma_start(out=outT_d[row : row + P, :], in_=o_t[:])
            ps_o_cm.__exit__(None, None, None)
            ps_e_cm.__exit__(None, None, None)

    _fix_multi_waits(nc)
    return nc


_NC_CACHE = None
LAST_EXEC_TIME_NS = None
LAST_RESULTS = None


def _get_nc():
    global _NC_CACHE
    if _NC_CACHE is None:
        _NC_CACHE = _build_nc()
    return _NC_CACHE


def kernel(**inputs) -> np.ndarray:
    global LAST_EXEC_TIME_NS, LAST_RESULTS
    x1 = np.asarray(inputs["x1"], np.float32)
    x2 = np.asarray(inputs["x2"], np.float32)

    bf16 = mybir.dt.np(BF16)
    x1f = np.ascontiguousarray(x1.reshape(B, C, N))
    x2f = np.ascontiguousarray(x2.reshape(B, C, N))
    x1b = x1f.astype(bf16)
    x2b = x2f.astype(bf16)

    branch_w = []
    for r in (1, 2):
        wq = np.asarray(inputs[f"wq{r}"], np.float32)
        wk = np.asarray(inputs[f"wk{r}"], np.float32)
        wv = np.asarray(inputs[f"wv{r}"], np.float32)
        branch_w.append(
            dict(
                wqT=np.ascontiguousarray(wq.T).astype(bf16),
                wkT=np.ascontiguousarray(wk.T).astype(bf16),
                wvT=np.ascontiguousarray(wv.T).astype(bf16),
                bq=np.ascontiguousarray(
                    np.asarray(inputs[f"bq{r}"], np.float32).reshape(D, 1)
                ),
                bk=np.ascontiguousarray(
                    np.asarray(inputs[f"bk{r}"], np.float32).reshape(D, 1)
                ),
                bv=np.ascontiguousarray(
                    np.asarray(inputs[f"bv{r}"], np.float32).reshape(1, C)
                ),
            )
        )

    in_maps = []
    for core in range(8):
        r = core // B
        b = core % B
        m = dict(branch_w[r])
        m["xq"] = x1b[b]
        m["xkv"] = x2b[b]
        in_maps.append(m)

    nc = _get_nc()

    trace = os.environ.get("KERNEL_TRACE") == "1"
    res = run_bass_kernel_spmd(nc, in_maps, list(range(8)), trace=trace)
    LAST_EXEC_TIME_NS = res.exec_time_ns
    LAST_RESULTS = res

    out = np.empty((B, C, N), np.float32)
    for b in range(B):
        out[b] = (
            x1f[b]
            + x2f[b]
            + res.results[b]["outT"].T
            + res.results[b + 4]["outT"].T
        )
    return out.reshape(B, C, H, W)
